# revision 13
# baseline (speedup 1.0000x reference)
"""Trainium2 Bass kernel for nn_MMN_34995393527847 (2D-TAN-style moment map network).

Math (per video b):
  map2d_X[j,n,m] = sum_d X[j,d] f[d,n] f[d,m]          (X in {Wiou, Wp})
  iou          = sigmoid(10 * <l2n(sent_iou), l2n_j(map2d_iou)>) * triu
  contrastive  =             <l2n(sent),     l2n_j(map2d_p)>    * triu
  fusion       = sigmoid(10 * sum_k Wfuse_k (v1[k,n] q2[s,k]) (v1[k,m] q2[s,k])) * triu

Key restructure vs the straightforward kernel:
  * Triangle packing: the output is triu-masked, and map2d is symmetric in
    (n,m), so only the 2080 pairs with m>=n are computed. They are packed
    band-major (band k = diagonal m-n = k), padded to 2176 = 17 chunks of
    128 pairs. Bands are built on VectorE/GpSimd as shifted elementwise
    products H[d, off_k+n] = f[d,n]*f[d,n+k].
  * QR trick: for each video and head, QR-factorize the l2-normalized
    sentence matrix L^T = Q R.  With W' = Q^T W, rows 0..7 of M' = W' h
    span all numerators (numer = R^T M'[0:8]) and ||M'|| = ||W h||.  So a
    single [128pair, 512] psum (256 j-cols per head) per chunk carries
    everything: no separate numerator matmuls at all.
  * Raw M'[0:8] columns are DMA'd straight from PSUM; per-pair norm^2
    accumulates via ScalarE Square+accum (iou head) and VectorE
    tensor_tensor_reduce (contrastive head) and is DMA'd out per video.
    Host does rsqrt-scale, R-combine, sigmoid and the (n,m) scatter.
  * Fusion head: one 512-wide matmul per video (out[n, (s,m)]).
  * f32->f32r via bitcast views (no CAST copies).

Sharding: data-parallel over B (16 videos -> 8 cores x 2). Weights replicated.
"""
import numpy as np

B, S, N, D, J, K = 16, 8, 64, 512, 256, 128
NCORES = 8
BPC = B // NCORES          # videos per core
DCH = D // 128             # 4 contraction chunks
NPAIR = (N * (N + 1)) // 2  # 2080 valid (m>=n) pairs
NCH = 17                   # chunks of 128 pairs (2176 slots, 96 pad)
NPAD = NCH * 128
VEC_KMAX = 32              # bands k<32 built on VectorE, k>=32 on GpSimd

# band k (m-n = k) starts at slot BOFF[k], length 64-k
BOFF = np.concatenate([[0], np.cumsum(64 - np.arange(64))]).astype(np.int64)

_cache = {}


def _build_program():
    from concourse import bacc, mybir, tile

    f32 = mybir.dt.float32
    f32r = mybir.dt.float32r

    nc = bacc.Bacc(None, target_bir_lowering=False)

    # per-core inputs
    feats_d = nc.declare_dram_parameter("fc", [128, BPC, DCH, N], f32r, isOutput=False)
    wq_d = nc.declare_dram_parameter("wq", [128, BPC, DCH, 2 * J], f32r, isOutput=False)
    w1_d = nc.declare_dram_parameter("w1c", [128, DCH, K], f32r, isOutput=False)
    b1_d = nc.declare_dram_parameter("b1t", [128, 1], f32, isOutput=False)
    cv_d = nc.declare_dram_parameter("cvecT", [128, BPC * S], f32, isOutput=False)

    # per-core outputs (raw, band-packed; host does the epilogue)
    sc_d = nc.declare_dram_parameter("sc", [BPC, 128, NCH, 16], f32, isOutput=True)
    nm_d = nc.declare_dram_parameter("nm", [BPC, 128, 2 * NCH], f32, isOutput=True)
    fu_d = nc.declare_dram_parameter("fu", [BPC, N, S * N], f32, isOutput=True)

    SIG = mybir.ActivationFunctionType.Sigmoid
    SQ = mybir.ActivationFunctionType.Square
    ADD = mybir.AluOpType.add

    with tile.TileContext(nc) as tc:
        with (
            tc.tile_pool(name="const", bufs=1) as cpool,
            tc.tile_pool(name="fsb", bufs=2) as fsb,
            tc.tile_pool(name="sscr", bufs=2) as sscr,
            tc.tile_pool(name="ps_mt", bufs=6, space="PSUM") as ps_mt,
            tc.tile_pool(name="ps_f", bufs=1, space="PSUM") as ps_f,
            tc.tile_pool(name="ps_v1", bufs=1, space="PSUM") as ps_v1,
        ):
            # ---- constants / inputs ----
            w1_t = cpool.tile([128, DCH, K], f32r, tag="w1")
            b1_t = cpool.tile([128, 1], f32, tag="b1")
            cv_t = cpool.tile([128, BPC * S], f32, tag="cv")
            f_t = cpool.tile([128, BPC, DCH, N], f32r, tag="f")
            wq_t = cpool.tile([128, BPC, DCH, 2 * J], f32r, tag="wq")
            H_t = cpool.tile([128, BPC, DCH, NPAD], f32r, tag="H")
            n2_t = cpool.tile([128, BPC, 2 * NCH], f32, tag="n2")
            scb_t = cpool.tile([128, BPC, NCH, 16], f32, tag="scb")

            nc.sync.dma_start(w1_t[:], w1_d[:])
            nc.sync.dma_start(b1_t[:], b1_d[:])
            nc.sync.dma_start(cv_t[:], cv_d[:])
            for v in range(BPC):
                nc.sync.dma_start(f_t[:, v], feats_d[:, v])
                nc.sync.dma_start(wq_t[:, v], wq_d[:, v])

            def fusion_branch(v):
                # v1 = W1^T F + b1   [K=128, N]
                v1_ps = ps_v1.tile([128, N], f32, tag="v1ps")
                for d in range(DCH):
                    nc.tensor.matmul(v1_ps[:], w1_t[:, d],
                                     f_t[:, v, d],
                                     start=(d == 0), stop=(d == DCH - 1))
                v1_t = fsb.tile([128, N], f32r, tag="v1")
                b1b = b1_t[:, 0:1].broadcast_to([128, N])
                nc.vector.tensor_add(v1_t[:], v1_ps[:], b1b)
                # z[k, s, m] = cvec[k, s] * v1[k, m]
                z_t = fsb.tile([128, S, N], f32r, tag="z")
                in0 = v1_t[:].unsqueeze(1).broadcast_to([128, S, N])
                in1 = cv_t[:, v * S:(v + 1) * S].unsqueeze(2).broadcast_to([128, S, N])
                nc.vector.tensor_mul(z_t[:], in0, in1)
                # fus[n, (s,m)] = sum_k v1[k,n] z[k,(s,m)]
                fus_ps = ps_f.tile([N, S * N], f32, tag="fps")
                nc.tensor.matmul(fus_ps[:], v1_t[:],
                                 z_t[:].rearrange("p s n -> p (s n)"),
                                 start=True, stop=True)
                fus_sb = fsb.tile([N, S * N], f32, tag="fsb")
                nc.scalar.activation(fus_sb[:], fus_ps[:], SIG, scale=10.0)
                nc.sync.dma_start(fu_d[v], fus_sb[:])

            def emit_band(v, k, eng):
                L = N - k
                off = int(BOFF[k])
                out = H_t[:, v, :, off:off + L]
                in0 = f_t[:, v, :, 0:L]
                in1 = f_t[:, v, :, k:N]
                eng.tensor_mul(out, in0, in1)

            def emit_chunk(v, c):
                mt = ps_mt.tile([128, 2 * J], f32, tag="mt")
                for d in range(DCH):
                    hsl = H_t[:, v, d, c * 128:(c + 1) * 128]
                    nc.tensor.matmul(mt[:], hsl, wq_t[:, v, d],
                                     start=(d == 0), stop=(d == DCH - 1))
                # raw numerator columns PSUM -> SBUF staging (ScalarE)
                src = mt[:].rearrange("p (h j) -> p h j", h=2)[:, :, 0:8]
                dst = scb_t[:, v, c].rearrange("p (h j) -> p h j", h=2)
                nc.scalar.copy(dst, src)
                # per-pair norms: ScalarE squares both heads PSUM->SBUF in one
                # op, VectorE reduces both heads in one op
                n2v = n2_t[:, v]
                sq = sscr.tile([128, 2 * J], f32, tag="sq")
                nc.scalar.activation(sq[:], mt[:], SQ)
                nc.vector.tensor_reduce(
                    out=n2v[:, 2 * c:2 * c + 2],
                    in_=sq[:].rearrange("p (h j) -> p h j", h=2),
                    axis=mybir.AxisListType.X, op=ADD)

            # ---- video 0: fusion, bands, then chunks (bands of video 1
            # interleaved into the chunk loop to keep VectorE fed) ----
            fusion_branch(0)
            for k in range(VEC_KMAX):
                emit_band(0, k, nc.vector)
            for k in range(VEC_KMAX, N):
                emit_band(0, k, nc.gpsimd)
            for v in range(BPC):
                nc.gpsimd.tensor_scalar_mul(
                    H_t[:, v, :, NPAIR:NPAD],
                    f_t[:, v, :, 0:1].broadcast_to([128, DCH, NPAD - NPAIR]), 0.0)
            for k in range(VEC_KMAX, N):
                emit_band(1, k, nc.gpsimd)

            vband = 0
            for c in range(NCH):
                emit_chunk(0, c)
                # two video-1 vector bands per chunk
                for _ in range(2):
                    if vband < VEC_KMAX:
                        emit_band(1, vband, nc.vector)
                        vband += 1
            while vband < VEC_KMAX:
                emit_band(1, vband, nc.vector)
                vband += 1
            nc.sync.dma_start(nm_d[0], n2_t[:, 0])
            nc.sync.dma_start(sc_d[0], scb_t[:, 0])

            fusion_branch(1)
            for c in range(NCH):
                emit_chunk(1, c)
            nc.sync.dma_start(nm_d[1], n2_t[:, 1])
            nc.sync.dma_start(sc_d[1], scb_t[:, 1])

    nc.compile()
    return nc


def _l2n(x):
    return x / np.clip(np.linalg.norm(x, axis=-1, keepdims=True), 1e-12, None)


def _prep_inputs(feats, sent_feat, sent_feat_iou, sent_feat_fusion,
                 W1, b1, W2, b2, Wp, Wiou, Wfuse):
    L_iou = _l2n(np.asarray(sent_feat_iou, np.float64))   # [B,S,J]
    L_p = _l2n(np.asarray(sent_feat, np.float64))
    Wiou64 = np.asarray(Wiou, np.float64)
    Wp64 = np.asarray(Wp, np.float64)

    wq_all = np.empty((B, 2 * J, D), np.float32)
    R_iou = np.empty((B, S, S), np.float64)
    R_p = np.empty((B, S, S), np.float64)
    for b in range(B):
        Qi, Ri = np.linalg.qr(L_iou[b].T, mode='complete')   # [J,J], [J,S]
        Qp, Rp = np.linalg.qr(L_p[b].T, mode='complete')
        R_iou[b] = Ri[:S, :]
        R_p[b] = Rp[:S, :]
        wq_all[b, :J] = (Qi.T @ Wiou64).astype(np.float32)
        wq_all[b, J:] = (Qp.T @ Wp64).astype(np.float32)

    q2 = sent_feat_fusion @ W2 + b2                 # [B,S,K]
    cvec = Wfuse[None, None, :] * q2 * q2           # [B,S,K]

    w1c = np.ascontiguousarray(
        W1.reshape(DCH, 128, K).transpose(1, 0, 2)).astype(np.float32)
    b1t = b1.reshape(128, 1).astype(np.float32)

    in_maps = []
    for core in range(NCORES):
        bs = slice(core * BPC, (core + 1) * BPC)
        fc = np.ascontiguousarray(
            feats[bs].reshape(BPC, DCH, 128, N).transpose(2, 0, 1, 3)).astype(np.float32)
        wq = np.ascontiguousarray(
            wq_all[bs].reshape(BPC, 2 * J, DCH, 128).transpose(3, 0, 2, 1)).astype(np.float32)
        cvT = np.ascontiguousarray(
            cvec[bs].transpose(2, 0, 1).reshape(K, BPC * S)).astype(np.float32)
        in_maps.append({
            "fc": fc, "wq": wq, "w1c": w1c, "b1t": b1t, "cvecT": cvT,
        })
    return in_maps, R_iou, R_p


def _sigmoid(x):
    out = np.empty_like(x)
    pos = x >= 0
    out[pos] = 1.0 / (1.0 + np.exp(-x[pos]))
    ex = np.exp(x[~pos])
    out[~pos] = ex / (1.0 + ex)
    return out


# slot -> (n, m) mapping for the band-packed pair layout
_SN = np.empty(NPAIR, np.int64)
_SM = np.empty(NPAIR, np.int64)
for _k in range(N):
    _sl = slice(int(BOFF[_k]), int(BOFF[_k + 1]))
    _SN[_sl] = np.arange(N - _k)
    _SM[_sl] = np.arange(N - _k) + _k


def _assemble(results, R_iou, R_p):
    iou = np.zeros((B, S, N, N), np.float32)
    con = np.zeros((B, S, N, N), np.float32)
    fus = np.empty((B, S, N, N), np.float32)
    triu = np.triu(np.ones((N, N), np.float32))
    for core, r in enumerate(results):
        for v in range(BPC):
            b = core * BPC + v
            raw = r["sc"][v].transpose(1, 0, 2).reshape(NPAD, 16)[:NPAIR].astype(np.float64)
            n2 = r["nm"][v].reshape(128, NCH, 2).transpose(1, 0, 2)
            n2 = n2.reshape(NPAD, 2)[:NPAIR].astype(np.float64)
            rn = 1.0 / np.maximum(np.sqrt(n2), 1e-12)
            iou_f = _sigmoid(10.0 * ((raw[:, 0:8] * rn[:, 0:1]) @ R_iou[b]))
            con_f = (raw[:, 8:16] * rn[:, 1:2]) @ R_p[b]
            iou[b][:, _SN, _SM] = iou_f.T.astype(np.float32)
            con[b][:, _SN, _SM] = con_f.T.astype(np.float32)
            fus[b] = r["fu"][v].reshape(N, S, N).transpose(1, 0, 2) * triu
    return np.stack([iou, fus, con], axis=0)


def _run(inputs, trace=False):
    from concourse.bass_utils import run_bass_kernel_spmd
    if "nc" not in _cache:
        _cache["nc"] = _build_program()
    in_maps, R_iou, R_p = _prep_inputs(**inputs)
    res = run_bass_kernel_spmd(_cache["nc"], in_maps, list(range(NCORES)),
                               trace=trace)
    out = _assemble(res.results, R_iou, R_p)
    return out, res


def kernel(**inputs):
    out, _ = _run(inputs, trace=False)
    return out


# revision 14
# speedup vs baseline: 1.1555x; 1.1555x over previous
"""Trainium2 Bass kernel for nn_MMN_34995393527847 (2D-TAN-style moment map network).

Math (per video b):
  map2d_X[j,n,m] = sum_d X[j,d] f[d,n] f[d,m]          (X in {Wiou, Wp})
  iou          = sigmoid(10 * <l2n(sent_iou), l2n_j(map2d_iou)>) * triu
  contrastive  =             <l2n(sent),     l2n_j(map2d_p)>    * triu
  fusion       = sigmoid(10 * sum_k Wfuse_k (v1[k,n] q2[s,k]) (v1[k,m] q2[s,k])) * triu

Key restructure vs the straightforward kernel:
  * Triangle packing: the output is triu-masked and map2d is symmetric in
    (n,m), so only pairs with m>=n are needed.  Pairs are packed in 8
    row-bands (band I = rows 8I..8I+7 x cols 8I..63), 2304 slots = 18
    chunks of 128 pairs (vs 32 chunks for the full square).  Each band
    builds as ONE dense contiguous elementwise op H[d,slot]=f[d,n]*f[d,m]
    on VectorE/GpSimd (dense APs run ~3x faster than per-diagonal ragged
    ones).  Sub-diagonal slots inside a band are computed but dropped by
    the host.
  * QR trick: for each video and head, QR-factorize the l2-normalized
    sentence matrix L^T = Q R.  With W' = Q^T W, rows 0..7 of M' = W' h
    span all numerators (numer = R^T M'[0:8]) and ||M'|| = ||W h||.  So a
    single [128pair, 512] psum (256 j-cols per head) per chunk carries
    everything: no separate numerator matmuls at all.
  * Per chunk: ScalarE squares the psum into SBUF ([128,512], one op) and
    copies the 16 raw numerator columns; VectorE reduces both heads' norms
    in one op.  Raw columns + norm^2 are DMA'd out; the host does
    rsqrt-scale, R-combine, sigmoid and the (n,m) scatter.
  * Fusion head: one 512-wide matmul per video (out[n, (s,m)]).
  * Matmul operands are f32r straight from DRAM (DMA into f32r tiles) or
    produced as f32r by the on-chip elementwise ops; no CAST copies.

Sharding: data-parallel over B (16 videos -> 8 cores x 2). Weights replicated.
"""
import numpy as np

B, S, N, D, J, K = 16, 8, 64, 512, 256, 128
NCORES = 8
BPC = B // NCORES          # videos per core
DCH = D // 128             # 4 contraction chunks
RB = 8                     # rows per band
NBAND = N // RB            # 8 bands
WID = [N - RB * I for I in range(NBAND)]          # band widths
BSZ = [RB * w for w in WID]                       # slots per band
ROFF = np.concatenate([[0], np.cumsum(BSZ)]).astype(np.int64)
NPAD = int(ROFF[-1])       # 2304 slots
NCH = NPAD // 128          # 18 chunks of 128 pairs

_cache = {}


def _build_program():
    from concourse import bacc, mybir, tile

    f32 = mybir.dt.float32
    f32r = mybir.dt.float32r

    nc = bacc.Bacc(None, target_bir_lowering=False)

    # per-core inputs
    feats_d = nc.declare_dram_parameter("fc", [128, BPC, DCH, N], f32r, isOutput=False)
    wq_d = nc.declare_dram_parameter("wq", [128, BPC, DCH, 2 * J], f32r, isOutput=False)
    w1_d = nc.declare_dram_parameter("w1c", [128, DCH, K], f32r, isOutput=False)
    b1_d = nc.declare_dram_parameter("b1t", [128, 1], f32, isOutput=False)
    cv_d = nc.declare_dram_parameter("cvecT", [128, BPC * S], f32, isOutput=False)

    # per-core outputs (raw, band-packed; host does the epilogue)
    sc_d = nc.declare_dram_parameter("sc", [BPC, 128, NCH, 16], f32, isOutput=True)
    nm_d = nc.declare_dram_parameter("nm", [BPC, 128, 2 * NCH], f32, isOutput=True)
    fu_d = nc.declare_dram_parameter("fu", [BPC, N, S * N], f32, isOutput=True)

    SIG = mybir.ActivationFunctionType.Sigmoid
    SQ = mybir.ActivationFunctionType.Square
    ADD = mybir.AluOpType.add

    with tile.TileContext(nc) as tc:
        with (
            tc.tile_pool(name="const", bufs=1) as cpool,
            tc.tile_pool(name="fsb", bufs=2) as fsb,
            tc.tile_pool(name="sscr", bufs=2) as sscr,
            tc.tile_pool(name="ps_mt", bufs=6, space="PSUM") as ps_mt,
            tc.tile_pool(name="ps_f", bufs=1, space="PSUM") as ps_f,
            tc.tile_pool(name="ps_v1", bufs=1, space="PSUM") as ps_v1,
        ):
            # ---- constants / inputs (order = DMA priority) ----
            w1_t = cpool.tile([128, DCH, K], f32r, tag="w1")
            b1_t = cpool.tile([128, 1], f32, tag="b1")
            cv_t = cpool.tile([128, BPC * S], f32, tag="cv")
            f_t = cpool.tile([128, BPC, DCH, N], f32r, tag="f")
            wq_t = cpool.tile([128, BPC, DCH, 2 * J], f32r, tag="wq")
            H_t = cpool.tile([128, BPC, DCH, NPAD], f32r, tag="H")
            n2_t = cpool.tile([128, BPC, 2 * NCH], f32, tag="n2")
            scb_t = cpool.tile([128, BPC, NCH, 16], f32, tag="scb")

            nc.sync.dma_start(w1_t[:], w1_d[:])
            nc.sync.dma_start(b1_t[:], b1_d[:])
            nc.sync.dma_start(cv_t[:], cv_d[:])
            for v in range(BPC):
                nc.sync.dma_start(f_t[:, v], feats_d[:, v])
            for v in range(BPC):
                for d in range(DCH):
                    nc.sync.dma_start(wq_t[:, v, d], wq_d[:, v, d])

            def fusion_branch(v):
                # v1 = W1^T F + b1   [K=128, N]
                v1_ps = ps_v1.tile([128, N], f32, tag="v1ps")
                for d in range(DCH):
                    nc.tensor.matmul(v1_ps[:], w1_t[:, d], f_t[:, v, d],
                                     start=(d == 0), stop=(d == DCH - 1))
                v1_t = fsb.tile([128, N], f32r, tag="v1")
                b1b = b1_t[:, 0:1].broadcast_to([128, N])
                nc.vector.tensor_add(v1_t[:], v1_ps[:], b1b)
                # z[k, s, m] = cvec[k, s] * v1[k, m]
                z_t = fsb.tile([128, S, N], f32r, tag="z")
                in0 = v1_t[:].unsqueeze(1).broadcast_to([128, S, N])
                in1 = cv_t[:, v * S:(v + 1) * S].unsqueeze(2).broadcast_to([128, S, N])
                nc.vector.tensor_mul(z_t[:], in0, in1)
                # fus[n, (s,m)] = sum_k v1[k,n] z[k,(s,m)]
                fus_ps = ps_f.tile([N, S * N], f32, tag="fps")
                nc.tensor.matmul(fus_ps[:], v1_t[:],
                                 z_t[:].rearrange("p s n -> p (s n)"),
                                 start=True, stop=True)
                fus_sb = fsb.tile([N, S * N], f32, tag="fsb")
                nc.scalar.activation(fus_sb[:], fus_ps[:], SIG, scale=10.0)
                nc.sync.dma_start(fu_d[v], fus_sb[:])

            def emit_band(v, I, eng):
                # H[:, v, d, ROFF[I] + r*WID + (m-8I)] = f[d, 8I+r] * f[d, m]
                w = WID[I]
                out = H_t[:, v, :, int(ROFF[I]):int(ROFF[I + 1])]
                out = out.rearrange("p d (r m) -> p d r m", r=RB)
                fm = f_t[:, v, :, RB * I:N].unsqueeze(2).broadcast_to(
                    [128, DCH, RB, w])
                fn = f_t[:, v, :, RB * I:RB * I + RB].unsqueeze(3).broadcast_to(
                    [128, DCH, RB, w])
                eng.tensor_mul(out, fn, fm)

            def emit_chunk(v, c):
                mt = ps_mt.tile([128, 2 * J], f32, tag="mt")
                for d in range(DCH):
                    hsl = H_t[:, v, d, c * 128:(c + 1) * 128]
                    nc.tensor.matmul(mt[:], hsl, wq_t[:, v, d],
                                     start=(d == 0), stop=(d == DCH - 1))
                # raw numerator columns PSUM -> SBUF staging (ScalarE)
                src = mt[:].rearrange("p (h j) -> p h j", h=2)[:, :, 0:8]
                dst = scb_t[:, v, c].rearrange("p (h j) -> p h j", h=2)
                nc.scalar.copy(dst, src)
                # norms: ScalarE squares both heads PSUM->SBUF in one op,
                # VectorE reduces both heads in one op
                sq = sscr.tile([128, 2 * J], f32, tag="sq")
                nc.scalar.activation(sq[:], mt[:], SQ)
                nc.vector.tensor_reduce(
                    out=n2_t[:, v, 2 * c:2 * c + 2],
                    in_=sq[:].rearrange("p (h j) -> p h j", h=2),
                    axis=mybir.AxisListType.X, op=ADD)

            # ---- emission: fusion & bands for v0, chunks v0 (v1 bands
            # interleaved), fusion & chunks v1 ----
            fusion_branch(0)
            emit_band(0, 0, nc.vector)
            for I in range(1, NBAND):
                emit_band(0, I, nc.gpsimd)
            for I in range(1, NBAND):
                emit_band(1, I, nc.gpsimd)

            for c in range(NCH):
                emit_chunk(0, c)
                if c == 1:
                    emit_band(1, 0, nc.vector)
            nc.sync.dma_start(nm_d[0], n2_t[:, 0])
            nc.sync.dma_start(sc_d[0], scb_t[:, 0])

            fusion_branch(1)
            for c in range(NCH):
                emit_chunk(1, c)
            nc.sync.dma_start(nm_d[1], n2_t[:, 1])
            nc.sync.dma_start(sc_d[1], scb_t[:, 1])

    nc.compile()
    return nc


def _l2n(x):
    return x / np.clip(np.linalg.norm(x, axis=-1, keepdims=True), 1e-12, None)


def _prep_inputs(feats, sent_feat, sent_feat_iou, sent_feat_fusion,
                 W1, b1, W2, b2, Wp, Wiou, Wfuse):
    L_iou = _l2n(np.asarray(sent_feat_iou, np.float64))   # [B,S,J]
    L_p = _l2n(np.asarray(sent_feat, np.float64))
    Wiou64 = np.asarray(Wiou, np.float64)
    Wp64 = np.asarray(Wp, np.float64)

    wq_all = np.empty((B, 2 * J, D), np.float32)
    R_iou = np.empty((B, S, S), np.float64)
    R_p = np.empty((B, S, S), np.float64)
    for b in range(B):
        Qi, Ri = np.linalg.qr(L_iou[b].T, mode='complete')   # [J,J], [J,S]
        Qp, Rp = np.linalg.qr(L_p[b].T, mode='complete')
        R_iou[b] = Ri[:S, :]
        R_p[b] = Rp[:S, :]
        wq_all[b, :J] = (Qi.T @ Wiou64).astype(np.float32)
        wq_all[b, J:] = (Qp.T @ Wp64).astype(np.float32)

    q2 = sent_feat_fusion @ W2 + b2                 # [B,S,K]
    cvec = Wfuse[None, None, :] * q2 * q2           # [B,S,K]

    w1c = np.ascontiguousarray(
        W1.reshape(DCH, 128, K).transpose(1, 0, 2)).astype(np.float32)
    b1t = b1.reshape(128, 1).astype(np.float32)

    in_maps = []
    for core in range(NCORES):
        bs = slice(core * BPC, (core + 1) * BPC)
        fc = np.ascontiguousarray(
            feats[bs].reshape(BPC, DCH, 128, N).transpose(2, 0, 1, 3)).astype(np.float32)
        wq = np.ascontiguousarray(
            wq_all[bs].reshape(BPC, 2 * J, DCH, 128).transpose(3, 0, 2, 1)).astype(np.float32)
        cvT = np.ascontiguousarray(
            cvec[bs].transpose(2, 0, 1).reshape(K, BPC * S)).astype(np.float32)
        in_maps.append({
            "fc": fc, "wq": wq, "w1c": w1c, "b1t": b1t, "cvecT": cvT,
        })
    return in_maps, R_iou, R_p


def _sigmoid(x):
    out = np.empty_like(x)
    pos = x >= 0
    out[pos] = 1.0 / (1.0 + np.exp(-x[pos]))
    ex = np.exp(x[~pos])
    out[~pos] = ex / (1.0 + ex)
    return out


# slot -> (n, m) mapping for the row-band pair layout
_SN = np.empty(NPAD, np.int64)
_SM = np.empty(NPAD, np.int64)
for _I in range(NBAND):
    for _r in range(RB):
        _s = int(ROFF[_I]) + _r * WID[_I]
        _SN[_s:_s + WID[_I]] = RB * _I + _r
        _SM[_s:_s + WID[_I]] = np.arange(RB * _I, N)
_VAL = _SM >= _SN
_VN = _SN[_VAL]
_VM = _SM[_VAL]


def _assemble(results, R_iou, R_p):
    iou = np.zeros((B, S, N, N), np.float32)
    con = np.zeros((B, S, N, N), np.float32)
    fus = np.empty((B, S, N, N), np.float32)
    triu = np.triu(np.ones((N, N), np.float32))
    for core, r in enumerate(results):
        for v in range(BPC):
            b = core * BPC + v
            raw = r["sc"][v].transpose(1, 0, 2).reshape(NPAD, 16)[_VAL].astype(np.float64)
            n2 = r["nm"][v].reshape(128, NCH, 2).transpose(1, 0, 2)
            n2 = n2.reshape(NPAD, 2)[_VAL].astype(np.float64)
            rn = 1.0 / np.maximum(np.sqrt(n2), 1e-12)
            iou_f = _sigmoid(10.0 * ((raw[:, 0:8] * rn[:, 0:1]) @ R_iou[b]))
            con_f = (raw[:, 8:16] * rn[:, 1:2]) @ R_p[b]
            iou[b][:, _VN, _VM] = iou_f.T.astype(np.float32)
            con[b][:, _VN, _VM] = con_f.T.astype(np.float32)
            fus[b] = r["fu"][v].reshape(N, S, N).transpose(1, 0, 2) * triu
    return np.stack([iou, fus, con], axis=0)


def _run(inputs, trace=False):
    from concourse.bass_utils import run_bass_kernel_spmd
    if "nc" not in _cache:
        _cache["nc"] = _build_program()
    in_maps, R_iou, R_p = _prep_inputs(**inputs)
    res = run_bass_kernel_spmd(_cache["nc"], in_maps, list(range(NCORES)),
                               trace=trace)
    out = _assemble(res.results, R_iou, R_p)
    return out, res


def kernel(**inputs):
    out, _ = _run(inputs, trace=False)
    return out
